# revision 1
# baseline (speedup 1.0000x reference)
"""MoE routing kernel for Trainium2 (8 NeuronCores, data-parallel over batch).

Stage-1 matmul in fp16 (4x PE rate vs fp32, half the HBM bytes
for the dominant x^T stream, pre-converted host-side); gather stream
(pN) and top-8 mixing weights also fp16 (adds ~5e-4 relative error,
negligible vs the fp16 routing noise). Softmax/top-8 selection stays
f32. 8-chunk DMA groups give 2-4KB contiguous runs per partition.
Tile mix (256,256,256,128,128) spreads the W1 load crunch and the
per-tile finish chains evenly (measured -14us vs a 512-wide lead tile).
Output stores are emitted at the end of the sync queue so they never
block the gpsimd gather queue. Each tile's stage-2/3 + routing is
deferred until the next tile's first stage-1 groups are enqueued, so
the in-order PE queue never stalls on the scalar relu at tile
boundaries. (A fused multi-offset indirect gather was tried and
produced garbage + was slower: the HW indirect DMA path really does
consume one offset per partition per descriptor. Keep 8 per-slot
gathers, pairwise-interleaved with the weighted sum.)

Pipeline per core (batch shard of 1024 rows):
  h1^T = relu(W1^T @ flat^T + b1)   # contraction D=16384, fp16 on PE
  h2^T = relu(W2^T @ h1^T + b2)
  logits = h2^T.T @ W3 + b3         # [128b, 64e] tiles
  s = softmax(logits) in f32; top-8 via DVE max/max_index;
  weights = top_vals / sum(top_vals)
  gather selected expert rows via indirect DMA; weighted sum; store.

Host-side layout: pTr[p, (t,g,cc,b)] so each (tile, k-group) DMA is a
single [128 x kper*nt] straight copy with kper*nt*2B contiguous runs
per partition (4 KB for the 256-wide tiles).
"""

import numpy as np

B, E, C, TOPK = 8192, 64, 256, 8
D, H1, H2 = 16384, 256, 128
NCORES = 8
BS = B // NCORES  # batch rows per core
P = 128
KPER = 8
TILES = (128, 256, 256, 256, 128)

_BUILD_CACHE = {}


def build_moe_nc(bs=BS, d=D, e=E, c=C, h1=H1, h2=H2, tiles=TILES, kper=KPER,
                 xbufs=12, hbufs=3, fused_gather=False, debug_taps=False):
    import concourse.bacc as bacc
    import concourse.bass as bass
    import concourse.mybir as mybir
    from concourse import tile

    f32 = mybir.dt.float32
    f16 = mybir.dt.float16
    u32 = mybir.dt.uint32
    KC = d // P            # 128-row K-chunks in main contraction
    KG = KC // kper        # DMA groups of kper chunks
    MC = h1 // P           # output row chunks of h1^T
    assert sum(tiles) == bs
    KC2 = h1 // P          # K-chunks for stage 2
    NBT = bs // P          # total 128-row batch subtiles

    nc = bacc.Bacc("TRN2", target_bir_lowering=False, debug=False,
                   num_devices=NCORES)

    pTr = nc.dram_tensor("pTr", [P, KC * bs], f16, kind="ExternalInput").ap()
    pN = nc.dram_tensor("pN", [bs * e, c], f16, kind="ExternalInput").ap()
    w1r = nc.dram_tensor("w1r", [P, KC * h1], f16, kind="ExternalInput").ap()
    w2r = nc.dram_tensor("w2r", [P, KC2 * h2], f32, kind="ExternalInput").ap()
    w3 = nc.dram_tensor("w3", [h2, e], f32, kind="ExternalInput").ap()
    b1r = nc.dram_tensor("b1r", [P, MC], f32, kind="ExternalInput").ap()
    b2r = nc.dram_tensor("b2r", [P, 1], f32, kind="ExternalInput").ap()
    b3r = nc.dram_tensor("b3r", [P, e], f32, kind="ExternalInput").ap()
    out = nc.dram_tensor("out", [bs, c], f16, kind="ExternalOutput").ap()
    if debug_taps:
        dbg_lg = nc.dram_tensor("dbg_lg", [bs, e], f32, kind="ExternalOutput").ap()
        dbg_ti = nc.dram_tensor("dbg_ti", [bs, 8], u32, kind="ExternalOutput").ap()
        dbg_w = nc.dram_tensor("dbg_w", [bs, 8], f32, kind="ExternalOutput").ap()
        dbg_sS = nc.dram_tensor("dbg_sS", [bs, e], f32, kind="ExternalOutput").ap()

    AF = mybir.ActivationFunctionType
    OP = mybir.AluOpType

    with tile.TileContext(nc) as tc:
        with (
            tc.tile_pool(name="wconst", bufs=1) as wconst,
            tc.tile_pool(name="w1pool", bufs=1) as w1pool,
            tc.tile_pool(name="xpool", bufs=xbufs) as xpool,
            tc.tile_pool(name="hpool", bufs=hbufs) as hpool,
            tc.tile_pool(name="spool", bufs=6) as spool,
            tc.tile_pool(name="selpool", bufs=4) as selpool,
            tc.tile_pool(name="opool", bufs=1) as opool,
            tc.tile_pool(name="psh1", bufs=2, space="PSUM") as psh1,
            tc.tile_pool(name="psh2", bufs=2, space="PSUM") as psh2,
            tc.tile_pool(name="pslg", bufs=2, space="PSUM") as pslg,
        ):
            # --- constants (small); emitted AFTER the first k-group DMAs so
            # they don't delay the PE-critical xt/W1 stream at startup
            cst = {}
            rb_tiles = []

            def emit_consts():
                cst["w2"] = wconst.tile([P, KC2 * h2], f32, name="w2_sb")
                nc.scalar.dma_start(out=cst["w2"], in_=w2r)
                cst["w3"] = wconst.tile([P, e], f32, name="w3_sb")
                nc.scalar.dma_start(out=cst["w3"][:h2, :], in_=w3)
                cst["b1"] = wconst.tile([P, MC], f32, name="b1_sb")
                nc.scalar.dma_start(out=cst["b1"], in_=b1r)
                cst["b2"] = wconst.tile([P, 1], f32, name="b2_sb")
                nc.scalar.dma_start(out=cst["b2"], in_=b2r)
                cst["b3"] = wconst.tile([P, e], f32, name="b3_sb")
                nc.scalar.dma_start(out=cst["b3"], in_=b3r)
                # per-bt DRAM row bases: rb[p] = (bg*P + p) * e  (constants)
                for bg in range(NBT):
                    rb = wconst.tile([P, 1], u32, tag=f"rb_{bg}", name=f"rb_{bg}")
                    nc.gpsimd.iota(rb, pattern=[[0, 1]], base=bg * P * e,
                                   channel_multiplier=e)
                    rb_tiles.append(rb)

            # --- W1 group tiles: persistent, loaded just-in-time in n=0 loop
            w1_tiles = [None] * KG
            acc_tiles = []

            def finish_tile(nt, col0, ps_h1):
                # relu(h1^T + b1) -> SBUF (f32: stage 2/3 stay full precision,
                # they are <2% of PE time and halve the top-8 tie flips)
                h1r = []
                for m in range(MC):
                    hr = hpool.tile([P, nt], f32, tag=f"h1r_{m}", name=f"h1r_{m}")
                    nc.scalar.activation(hr, ps_h1[m], AF.Relu,
                                         bias=cst["b1"][:, m:m + 1])
                    h1r.append(hr)

                # h2^T
                ps_h2 = psh2.tile([P, nt], f32, tag="h2", name="ps_h2")
                for k2 in range(KC2):
                    nc.tensor.matmul(out=ps_h2[:h2, :],
                                     lhsT=cst["w2"][:, k2 * h2:(k2 + 1) * h2],
                                     rhs=h1r[k2], start=(k2 == 0),
                                     stop=(k2 == KC2 - 1))
                h2r = hpool.tile([P, nt], f32, tag="h2r", name="h2r")
                nc.scalar.activation(h2r[:h2, :], ps_h2[:h2, :], AF.Relu,
                                     bias=cst["b2"][:h2, :])

                for bt in range(nt // P):
                    bg = col0 // P + bt  # global 128-row batch subtile index
                    ps_lg = pslg.tile([P, e], f32, tag="lg", name="ps_lg")
                    nc.tensor.matmul(out=ps_lg, lhsT=h2r[:h2, bt * P:(bt + 1) * P],
                                     rhs=cst["w3"][:h2, :], start=True, stop=True)
                    lg = spool.tile([P, e], f32, tag="lg_sb", name="lg_sb")
                    nc.vector.tensor_tensor(out=lg, in0=ps_lg, in1=cst["b3"], op=OP.add)

                    # f32 softmax, replicating the reference's quantization
                    nm = spool.tile([P, 1], f32, tag="nm", name="nm")
                    nc.vector.reduce_max(out=nm, in_=lg, axis=mybir.AxisListType.X,
                                         negate=True)
                    ef = spool.tile([P, e], f32, tag="ef", name="ef")
                    nc.scalar.activation(ef, lg, AF.Exp, bias=nm)
                    # top-8 straight on the unnormalized exps: selection
                    # order is scale-invariant (measured #8/#9 gaps are ~40x
                    # above f32 ulp) and tv/sum(tv) below cancels the scale,
                    # so the softmax normalization drops off the chain
                    tv = spool.tile([P, 8], f32, tag="tv", name="tv")
                    nc.vector.max(out=tv, in_=ef)
                    ti = spool.tile([P, 8], u32, tag="ti", name="ti")
                    nc.vector.max_index(out=ti, in_max=tv, in_values=ef)

                    s8 = spool.tile([P, 1], f32, tag="s8", name="s8")
                    nc.vector.reduce_sum(out=s8, in_=tv, axis=mybir.AxisListType.X)
                    r8 = spool.tile([P, 1], f32, tag="r8", name="r8")
                    nc.vector.reciprocal(r8, s8)
                    wgt = spool.tile([P, 8], f16, tag="wgt", name="wgt")
                    nc.scalar.activation(wgt, tv, AF.Copy, scale=r8)

                    # DRAM row index = (bg*128 + p)*e + expert
                    ridx = spool.tile([P, 8], u32, tag="ridx", name="ridx")
                    nc.vector.tensor_tensor(out=ridx, in0=ti,
                                            in1=rb_tiles[bg].to_broadcast([P, 8]),
                                            op=OP.add)

                    sel = selpool.tile([P, TOPK, c], f16, tag="sel", name="sel")
                    mt = selpool.tile([P, TOPK * c], f16, tag="mt", name="mt")
                    mt3 = mt.rearrange("p (k c) -> p k c", c=c)
                    wb = wgt.to_broadcast([P, TOPK, c])
                    if fused_gather:
                        # one multi-offset indirect DMA gathers all 8 selected
                        # expert rows per partition
                        nc.gpsimd.indirect_dma_start(
                            out=sel, out_offset=None, in_=pN,
                            in_offset=bass.IndirectOffsetOnAxis(ap=ridx, axis=0))
                        nc.vector.tensor_tensor(out=mt3, in0=sel, in1=wb,
                                                op=OP.mult)
                        for q in range(4):
                            nc.vector.tensor_add(
                                mt[:, 2 * q * c:(2 * q + 1) * c],
                                mt[:, 2 * q * c:(2 * q + 1) * c],
                                mt[:, (2 * q + 1) * c:(2 * q + 2) * c])
                    else:
                        for q in range(4):
                            ks = slice(2 * q, 2 * q + 2)
                            for kk in range(2 * q, 2 * q + 2):
                                nc.gpsimd.indirect_dma_start(
                                    out=sel[:, kk, :], out_offset=None, in_=pN,
                                    in_offset=bass.IndirectOffsetOnAxis(
                                        ap=ridx[:, kk:kk + 1], axis=0))
                            nc.vector.tensor_tensor(out=mt3[:, ks, :],
                                                    in0=sel[:, ks, :],
                                                    in1=wb[:, ks, :], op=OP.mult)
                            nc.vector.tensor_add(
                                mt[:, 2 * q * c:(2 * q + 1) * c],
                                mt[:, 2 * q * c:(2 * q + 1) * c],
                                mt[:, (2 * q + 1) * c:(2 * q + 2) * c])
                    nc.vector.tensor_add(mt[:, :c], mt[:, :c], mt[:, 2 * c:3 * c])
                    nc.vector.tensor_add(mt[:, 4 * c:5 * c], mt[:, 4 * c:5 * c],
                                         mt[:, 6 * c:7 * c])
                    acc = opool.tile([P, c], f16, tag=f"acc_{bg}", name=f"acc_{bg}")
                    nc.vector.tensor_add(acc, mt[:, :c], mt[:, 4 * c:5 * c])
                    acc_tiles.append((bg, acc))

                    if debug_taps:
                        rows = slice(bg * P, (bg + 1) * P)
                        lgc = spool.tile([P, e], f32, tag="lgc", name="lgc")
                        nc.vector.tensor_copy(out=lgc, in_=ps_lg)
                        nc.sync.dma_start(out=dbg_lg[rows, :], in_=lgc)
                        nc.sync.dma_start(out=dbg_ti[rows, :], in_=ti)
                        nc.sync.dma_start(out=dbg_w[rows, :], in_=wgt)

            nc.__enter_lp = nc.allow_low_precision(
                reason="fp16 weighted-sum tree of fp16 gathers; output "
                       "stores are fp16 regardless")
            nc.__enter_lp.__enter__()

            col0 = 0
            pending = None
            for n, nt in enumerate(tiles):
                toff = KC * col0  # column offset of this tile's block in pTr
                ps_h1 = [psh1.tile([P, nt], f32, tag=f"h1_{m}", name=f"ps_h1_{m}")
                         for m in range(MC)]
                for g in range(KG):
                    xt = xpool.tile([P, kper, nt], f16, tag="xt", name="xt")
                    if n == 0 and g == 0:
                        # per-chunk DMAs so the first matmul only waits for
                        # chunk 0 of xt and W1, not the whole group
                        wt = w1pool.tile([P, kper, h1], f16, tag="w1_0",
                                         name="w1_0")
                        w1_tiles[0] = wt
                        for cc in range(kper):
                            nc.sync.dma_start(
                                out=xt[:, cc, :],
                                in_=pTr[:, toff + cc * nt:toff + (cc + 1) * nt])
                            nc.sync.dma_start(
                                out=wt[:, cc, :],
                                in_=w1r[:, cc * h1:(cc + 1) * h1])
                    else:
                        nc.sync.dma_start(
                            out=xt,
                            in_=pTr[:, toff + g * kper * nt:
                                    toff + (g + 1) * kper * nt]
                            .rearrange("p (c b) -> p c b", c=kper))
                        if n == 0:
                            wt = w1pool.tile([P, kper, h1], f16, tag=f"w1_{g}",
                                             name=f"w1_{g}")
                            nc.sync.dma_start(
                                out=wt,
                                in_=w1r[:, g * kper * h1:(g + 1) * kper * h1]
                                .rearrange("p (c h) -> p c h", c=kper))
                            w1_tiles[g] = wt
                    wt = w1_tiles[g]
                    for cc in range(kper):
                        for m in range(MC):
                            nc.tensor.matmul(
                                out=ps_h1[m], lhsT=wt[:, cc, m * P:(m + 1) * P],
                                rhs=xt[:, cc, :],
                                start=(g == 0 and cc == 0),
                                stop=(g == KG - 1 and cc == kper - 1))
                    if n == 0 and g == 2:
                        emit_consts()
                    if g == 1 and pending is not None:
                        # previous tile's stage-2/3 + routing, enqueued only
                        # now so the PE queue never stalls on the relu at the
                        # tile boundary
                        finish_tile(*pending)
                        pending = None

                pending = (nt, col0, ps_h1)
                col0 += nt

            finish_tile(*pending)

            nc.__enter_lp.__exit__(None, None, None)

            # output stores, all at the tail of the in-order sync queue: by
            # the time the queue drains the xt stream, the early acc tiles
            # are long done, and nothing ever blocks the gpsimd gather queue
            for bg, acc in acc_tiles:
                nc.sync.dma_start(out=out[bg * P:(bg + 1) * P, :], in_=acc)

    nc.compile()
    return nc


def _prep_core_inputs(flat, W1b, w1r_, w2r_, w3_, b1, b2, b3, core,
                      tiles=TILES, kper=KPER):
    KC = D // P
    shard = flat[core * BS:(core + 1) * BS]                    # (BS, D)
    hf = shard.astype(np.float16)                              # (BS, D)
    blocks = []
    col0 = 0
    for nt in tiles:
        blk = hf[col0:col0 + nt, :].T                          # (D, nt)
        blocks.append(np.ascontiguousarray(
            blk.reshape(KC, P, nt).transpose(1, 0, 2).reshape(P, KC * nt)))
        col0 += nt
    pTr = np.concatenate(blocks, axis=1)                       # (P, KC*BS)
    pN = np.ascontiguousarray(hf).reshape(BS * E, C)
    b1r = np.ascontiguousarray(b1.reshape(H1 // P, P).T)
    b2r = np.ascontiguousarray(b2.reshape(H2, 1))
    b3r = np.ascontiguousarray(np.broadcast_to(b3, (P, E)))
    return {"pTr": pTr, "pN": pN, "w1r": w1r_, "w2r": w2r_, "w3": w3_,
            "b1r": b1r, "b2r": b2r, "b3r": b3r}


def _prep_weights(W1, W2, W3):
    KC = D // P
    w1r_ = np.ascontiguousarray(
        W1.astype(np.float16).reshape(KC, P, H1)
        .transpose(1, 0, 2).reshape(P, KC * H1))
    w2r_ = np.ascontiguousarray(
        W2.reshape(H1 // P, P, H2).transpose(1, 0, 2).reshape(P, -1))
    w3_ = np.ascontiguousarray(W3)
    return w1r_, w2r_, w3_


def kernel(expert_probs, W1, b1, W2, b2, W3, b3):
    from concourse.bass_utils import run_bass_kernel_spmd

    expert_probs = np.asarray(expert_probs, dtype=np.float32)
    W1 = np.asarray(W1, dtype=np.float32)
    W2 = np.asarray(W2, dtype=np.float32)
    W3 = np.asarray(W3, dtype=np.float32)
    b1 = np.asarray(b1, dtype=np.float32)
    b2 = np.asarray(b2, dtype=np.float32)
    b3 = np.asarray(b3, dtype=np.float32)

    if "nc" not in _BUILD_CACHE:
        _BUILD_CACHE["nc"] = build_moe_nc()
    nc = _BUILD_CACHE["nc"]

    flat = expert_probs.reshape(B, D)
    w1r_, w2r_, w3_ = _prep_weights(W1, W2, W3)
    in_maps = [_prep_core_inputs(flat, None, w1r_, w2r_, w3_, b1, b2, b3, cid)
               for cid in range(NCORES)]
    res = run_bass_kernel_spmd(nc, in_maps, core_ids=list(range(NCORES)))
    out = np.concatenate([res.results[cid]["out"] for cid in range(NCORES)], axis=0)
    return out.astype(np.float32)  # device stores fp16; ~1e-4 rel quantization

